# revision 6
# baseline (speedup 1.0000x reference)
"""Trainium2 Bass kernel for CosineSim3D.

Reference computation (per batch element b):
    a_mag[n] = sqrt(max(sum_d A[n,d]^2, eps))
    b_mag[m] = sqrt(max(sum_d B[m,d]^2, eps))
    scores[n] = sum_m (A[n,:] . B[m,:]) / (a_mag[n] * b_mag[m])
    probs = softmax(scores)
    out[n, :] = probs[n]  (tiled 300x)

Key algebraic collapse: the [n,m] similarity matrix is never needed --
    scores[n] = (A[n,:] . c) / a_mag[n],   c[d] = sum_m B[m,d] / b_mag[m]
which turns an O(n*m*d) batched matmul into O(n*d) work, making the
kernel DMA-bound (each core streams its full input/output shard).

Sharding: pure data parallel over the batch dim, 128 batches -> 8 cores
x 16 batches each.  Full inputs in, full output out; shard/gather here.
"""

import numpy as np

import concourse.bacc as bacc
import concourse.bass as bass
import concourse.tile as tile
from concourse import mybir
from concourse.bass_utils import run_bass_kernel_spmd

# Problem shape (hardcoded per contract)
B_FULL = 128
N = 1024          # rows per batch (both a and b)
D = 300           # feature dim
N_CORES = 8
B_SHARD = B_FULL // N_CORES   # 16 batches per core
P = 128           # SBUF partitions
C = N // P        # 8 row-chunks of 128 per batch
EPS = 1e-7

F32 = mybir.dt.float32
AF = mybir.ActivationFunctionType
ALU = mybir.AluOpType


def _build_program() -> bass.Bass:
    nc = bacc.Bacc(
        "TRN2",
        target_bir_lowering=False,
        debug=False,
        num_devices=N_CORES,
    )

    a_h = nc.declare_dram_parameter("a", [B_SHARD, N, D], F32, isOutput=False)
    b_h = nc.declare_dram_parameter("b", [B_SHARD, N, D], F32, isOutput=False)
    o_h = nc.declare_dram_parameter("out", [B_SHARD, N, D], F32, isOutput=True)

    # row index = p*C + c  ->  each partition holds C contiguous rows (9600 B)
    a_v = a_h[:].rearrange("s (p c) d -> s p c d", p=P)
    b_v = b_h[:].rearrange("s (p c) d -> s p c d", p=P)
    o_v = o_h[:].rearrange("s (p c) d -> s p c d", p=P)

    with tile.TileContext(nc) as tc:
        with (
            tc.tile_pool(name="singles", bufs=1) as singles,
            tc.tile_pool(name="big", bufs=3) as big,
            tc.tile_pool(name="mid", bufs=2) as mid,
            tc.tile_pool(name="small", bufs=3) as small,
            tc.tile_pool(name="psum", bufs=2, space="PSUM") as psum,
        ):
            ones_wide = singles.tile([P, D], F32, tag="ones_wide")
            nc.vector.memset(ones_wide, 1.0)
            ones_row = singles.tile([1, P], F32, tag="ones_row")
            nc.vector.memset(ones_row, 1.0)
            ones_col = singles.tile([P, 1], F32, tag="ones_col")
            nc.vector.memset(ones_col, 1.0)

            for i in range(B_SHARD):
                # ---- load shard batch i ----
                b_tile = big.tile([P, C, D], F32, tag="b_tile")
                nc.sync.dma_start(out=b_tile, in_=b_v[i])
                a_tile = big.tile([P, C, D], F32, tag="a_tile")
                nc.sync.dma_start(out=a_tile, in_=a_v[i])

                # ---- b row norms -> binv = 1/sqrt(max(ss, eps)) ----
                ssb = small.tile([P, C], F32, tag="ssb")
                sq_scr = mid.tile([P, D], F32, tag="sq_scr")
                for j in range(C):
                    nc.scalar.activation(
                        out=sq_scr,
                        in_=b_tile[:, j, :],
                        func=AF.Square,
                        accum_out=ssb[:, j : j + 1],
                    )
                binv = small.tile([P, C], F32, tag="binv")
                nc.vector.tensor_scalar_max(out=binv, in0=ssb, scalar1=EPS)
                nc.scalar.activation(out=binv, in_=binv, func=AF.Sqrt)
                nc.vector.reciprocal(out=binv, in_=binv)

                # ---- c[d] = sum_m B[m,d] * binv[m]  (PE partition-reduce) ----
                c_ps = psum.tile([1, D], F32, tag="c_ps")
                for j in range(C):
                    nc.tensor.matmul(
                        c_ps,
                        binv[:, j : j + 1],      # lhsT [K=128, M=1]
                        b_tile[:, j, :],         # rhs  [K=128, N=300]
                        start=(j == 0),
                        stop=(j == C - 1),
                    )
                c_sb = small.tile([1, D], F32, tag="c_sb")
                nc.scalar.copy(c_sb, c_ps)

                # broadcast c across partitions: ones[128,1(K)] x c[1(K),300]
                cb_ps = psum.tile([P, D], F32, tag="cb_ps")
                nc.tensor.matmul(cb_ps, ones_row, c_sb, start=True, stop=True)
                cb_sb = mid.tile([P, D], F32, tag="cb_sb")
                nc.scalar.copy(cb_sb, cb_ps)

                # ---- a row norms -> ainv (ACT square + accum) ----
                # (tensor_tensor_reduce crashes this runtime; see notes)
                ssa = small.tile([P, C], F32, tag="ssa")
                sq_scr_a = mid.tile([P, D], F32, tag="sq_scr_a")
                for j in range(C):
                    nc.scalar.activation(
                        out=sq_scr_a,
                        in_=a_tile[:, j, :],
                        func=AF.Square,
                        accum_out=ssa[:, j : j + 1],
                    )
                ainv = small.tile([P, C], F32, tag="ainv")
                nc.vector.tensor_scalar_max(out=ainv, in0=ssa, scalar1=EPS)
                nc.scalar.activation(out=ainv, in_=ainv, func=AF.Sqrt)
                nc.vector.reciprocal(out=ainv, in_=ainv)

                # ---- dot[n] = A[n,:] . c  (DVE mult then reduce) ----
                dot = small.tile([P, C], F32, tag="dot")
                dot_scr = mid.tile([P, D], F32, tag="dot_scr")
                for j in range(C):
                    nc.vector.tensor_mul(dot_scr, a_tile[:, j, :], cb_sb)
                    nc.vector.tensor_reduce(
                        out=dot[:, j : j + 1],
                        in_=dot_scr,
                        axis=mybir.AxisListType.X,
                        op=ALU.add,
                    )

                # scores = dot * ainv ; exp + per-partition row sums
                scores = small.tile([P, C], F32, tag="scores")
                nc.vector.tensor_mul(scores, dot, ainv)
                exp_s = small.tile([P, C], F32, tag="exp_s")
                row_sum = small.tile([P, 1], F32, tag="row_sum")
                nc.scalar.activation(
                    out=exp_s, in_=scores, func=AF.Exp, accum_out=row_sum
                )

                # Z = sum over partitions; invZ broadcast back to all rows
                z_ps = psum.tile([1, 1], F32, tag="z_ps")
                nc.tensor.matmul(z_ps, row_sum, ones_col, start=True, stop=True)
                inv_z = small.tile([1, 1], F32, tag="inv_z")
                nc.vector.reciprocal(out=inv_z, in_=z_ps)
                invz_ps = psum.tile([P, 1], F32, tag="invz_ps")
                nc.tensor.matmul(invz_ps, ones_row, inv_z, start=True, stop=True)
                invz_sb = small.tile([P, 1], F32, tag="invz_sb")
                nc.scalar.copy(invz_sb, invz_ps)

                probs = small.tile([P, C], F32, tag="probs")
                nc.vector.tensor_scalar_mul(out=probs, in0=exp_s, scalar1=invz_sb)

                # ---- expand probs -> [P, C, 300] and store ----
                out_tile = big.tile([P, C, D], F32, tag="out_tile")
                for j in range(C):
                    nc.vector.tensor_scalar_mul(
                        out=out_tile[:, j, :],
                        in0=ones_wide,
                        scalar1=probs[:, j : j + 1],
                    )
                nc.sync.dma_start(out=o_v[i], in_=out_tile)

    nc.finalize()
    return nc


_NC_CACHE = None


def _get_program():
    global _NC_CACHE
    if _NC_CACHE is None:
        _NC_CACHE = _build_program()
    return _NC_CACHE


def run(a: np.ndarray, b: np.ndarray, trace: bool = False):
    """Shard over batch, run on 8 cores, gather. Returns (out, BassKernelResults)."""
    a = np.ascontiguousarray(a, dtype=np.float32)
    b = np.ascontiguousarray(b, dtype=np.float32)
    assert a.shape == (B_FULL, N, D) and b.shape == (B_FULL, N, D)

    nc = _get_program()
    in_maps = [
        {
            "a": a[i * B_SHARD : (i + 1) * B_SHARD],
            "b": b[i * B_SHARD : (i + 1) * B_SHARD],
        }
        for i in range(N_CORES)
    ]
    res = run_bass_kernel_spmd(nc, in_maps, list(range(N_CORES)), trace=trace)
    out = np.concatenate([r["out"] for r in res.results], axis=0)
    return out, res


def kernel(a: np.ndarray, b: np.ndarray) -> np.ndarray:
    out, _ = run(a, b, trace=False)
    return out
